# revision 22
# baseline (speedup 1.0000x reference)
"""GQA causal attention (B=1, T=4096, D=1024, HQ=16, HKV=4, HD=64) on 8 trn2
NeuronCores via Bass/Tile.

Sharding: block-cyclic sequence-parallel. The 4096 query tokens are split into
64 blocks of 64; core i owns blocks {i, 8+i, ..., 56+i} (512 q tokens). Every
core runs the SAME program (SPMD): for its j-th block it processes k-tiles
[0, 4*(j+1)) — a core-independent conservative causal extent — and a
host-supplied per-core boundary mask zeroes the non-causal tail, so per-core
work is uniform AND balanced. K/V are projected redundantly per-core from a
bf16 copy of x^T (no collectives: an AllGather + its first-use barrier
measured ~85us of mostly-serial wall time, more than the ~50us of redundant
matmul work it saves).

Softmax exp is split across TWO engines so the ACT engine stops being the
pipeline bottleneck (it was ~160us of 363us):
  - ACT path: nc.scalar.activation(Exp) with the score scale folded into its
    free affine (scale=ln2/128, bias=ln K).
  - DVE path: a custom 8-stage vector op (EXP2B16_ANT) that computes the bf16
    BIT PATTERN of K*e^s directly: u = x + C1 (x = 128*s/ln2, folded into the
    Q cast scale); e = round-to-multiple-of-128(u) via the +/- 1.5*2^30 trick;
    f = u - e; bits = u + 0.316/128*f^2 + 53 (Schraudolph + quadratic mantissa
    correction), written as int16 and bitcast to bf16. Max rel err 0.59%, and
    the uniform factor K cancels in the softmax normalization ([V|1] rowsum
    uses the same pt values).
Boundary causal masks are applied by the otherwise-idle GPSIMD engine
(SBUF->SBUF bf16 multiplies by a 0/1 mask).

Layout strategy (avoids all on-device transposes):
  - scores are computed as S^T[k, q] with k on partitions, so the softmax
    denominator comes from a ones-column appended to the V stationary ([V|1]).
  - normalization is deferred: ctx^T/(rowsum) after the k-loop via a
    broadcast-matmul + fast reciprocal.
  - Wq/Wo columns/rows host-permuted so two heads stack into 128 partitions;
    score matmuls run pairwise-packed via tile_position row groups.
All weights/activations bf16 (FWL fast weight load + half the DMA/SBUF);
accumulation fp32 in PSUM.
"""

import math
import os
import sys

sys.path.insert(0, "/opt/trn_rl_repo")

import numpy as np
import ml_dtypes

import concourse.bass as bass
import concourse.bacc as bacc
import concourse.mybir as mybir
import concourse.tile as tile
from concourse.bass_utils import run_bass_kernel_spmd

# ---------------------------------------------------------------- constants
B, T, D = 1, 4096, 1024
HQ, HKV, HD = 16, 4, 64
G = HQ // HKV          # 4 q heads per kv head
NC = 8                 # cores
QB = 64                # q block size
NBLK = T // QB         # 64 blocks total
BPC = NBLK // NC       # 8 blocks per core
LQ = QB * BPC          # 512 local q tokens per core
DT = D // 128          # 8 contraction tiles over D
NKT = T // 128         # 32 k-tiles
F32 = mybir.dt.float32
F32R = mybir.dt.float32r
BF16 = mybir.dt.bfloat16
I16 = mybir.dt.int16
BF16NP = ml_dtypes.bfloat16

# exp split: scores arrive pre-scaled to x = 128*s/ln2 (folded into Q cast)
A128 = 128.0 / math.log(2.0)           # 184.665058...
QSCALE = A128 * (float(HD) ** -0.5)    # folded into the q bf16 cast
EXP_B128 = 16192.619648                # phase/level constant (calibrated)
EXP_C0 = -0.00247                      # -(mantissa curvature)/128
EXP_C3 = -22052.576                    # level decouple: C0*C3 = +54.47 bits
EXP_S128 = 1610612736.0                # 1.5*2^30: fp32 ulp=128 rounding trick
ACT_SCALE = math.log(2.0) / 128.0      # undoes the 128/ln2 prescale on ACT
ACT_BIAS = 0.009612216315190271        # ln K, K = DVE path's constant factor
DVE_K = int(os.environ.get("EXP_DVE_K", "3"))   # of every 8 groups -> DVE

# head pairing: pair tile m holds (LO[m] on partitions 0-63, HI[m] on 64-127)
LO = [0, 1, 2, 3, 8, 9, 10, 11]
HI = [4, 5, 6, 7, 12, 13, 14, 15]

# ------------------------------------------------- custom DVE exp operation
_EXP_OP = None


def _get_exp_op():
    """Register EXP2B16_ANT: out_bits16 = (u - ((f*f) - C3)*C0)*1 with
    u = in0 + C1, f = u - ((u + C2) - C2). Written as int16 (the bf16 bit
    pattern of K*e^s)."""
    global _EXP_OP
    if _EXP_OP is not None:
        return _EXP_OP
    from concourse.dve_spec import (
        C0, C1, C2, C3, Spec, Src0, lower, sq, _spill_c3_to_src1, _has_src1,
    )
    from concourse.dve_uop import DveOpSpec
    import concourse.dve_ops as dve_ops

    u = Src0 + C1
    w = u + C2
    e = w - C2
    f = u - e
    q = sq(f)
    r = q - C3
    p = r * C0
    body = u - p

    def _ref(in0, in1, s0, s1, imm2):
        # leaf binding: C0 <- s0 (curvature), C1 <- s1 (bias), C2 <- imm2,
        # C3 <- in1 (spilled, [P,1])
        x = in0.astype(np.float32)
        c3 = np.asarray(in1, np.float32).reshape(-1, *([1] * (x.ndim - 1)))
        uu = (x + np.float32(s1)).astype(np.float32)
        ee = ((uu + np.float32(imm2)) - np.float32(imm2)).astype(np.float32)
        ff = (uu - ee).astype(np.float32)
        pp = (((ff * ff) - c3) * np.float32(s0)).astype(np.float32)
        return (uu - pp).astype(np.float32)

    spec = Spec(body=_spill_c3_to_src1(body), reference=_ref)
    name = "EXP2B16_ANT"
    if name not in dve_ops._SUB_OPCODE_FOR_NAME:
        row = dve_ops._CUSTOM_DVE_ROW_BASE + len(dve_ops.OPS)
        dve_ops._SUB_OPCODE_FOR_NAME[name] = row
    shas = {}
    for ver in ("v3", "v4"):
        d = DveOpSpec(
            name=name,
            opcode=dve_ops._SUB_OPCODE_FOR_NAME[name],
            uops=lower(spec, ver=ver),
            rd1_en=_has_src1(spec),
        )
        shas[ver] = d.sha(ver)
    op = dve_ops.DveOp(name, spec, subdim=False, uops_sha=shas)
    if not any(o.name == name for o in dve_ops.OPS):
        dve_ops.OPS.append(op)
        dve_ops.CUSTOM_DVE_SPECS[name] = op.spec
    _EXP_OP = op
    return op


def _local_cols(i):
    """Global token indices owned by core i, in local order."""
    return np.concatenate(
        [QB * (NC * j + i) + np.arange(QB) for j in range(BPC)]
    )


def _band_mask(i):
    """[4, 128, 64] multiplicative causal mask for the last k-quartet of any
    block: valid iff 128*kt2 + p <= 64*i + f."""
    kt2 = np.arange(4)[:, None, None]
    p = np.arange(128)[None, :, None]
    f = np.arange(64)[None, None, :]
    return (128 * kt2 + p <= 64 * i + f).astype(BF16NP)


# ---------------------------------------------------------------- program
def build_nc():
    nc = bacc.Bacc(None)
    xo_d = nc.declare_dram_parameter("xT_own", [D, LQ], BF16, isOutput=False)
    xf_d = nc.declare_dram_parameter("xT_full", [D, T], BF16, isOutput=False)
    wq_d = nc.declare_dram_parameter("Wq_perm", [D, HQ * HD], BF16, isOutput=False)
    wk_d = nc.declare_dram_parameter("Wk_n", [D, HKV * HD], BF16, isOutput=False)
    wv_d = nc.declare_dram_parameter("Wv_n", [D, HKV * HD], BF16, isOutput=False)
    wo_d = nc.declare_dram_parameter("Wo_perm", [HQ * HD, D], BF16, isOutput=False)
    bm_d = nc.declare_dram_parameter("bmask", [4, 128, QB], BF16, isOutput=False)
    on_d = nc.declare_dram_parameter("ones_c", [1, HD], F32R, isOutput=False)
    out_d = nc.declare_dram_parameter("out_loc", [LQ, D], F32, isOutput=True)

    with tile.TileContext(nc) as tc:
        _emit(nc, tc, xo_d, xf_d, wq_d, wk_d, wv_d, wo_d, bm_d, on_d, out_d)
    nc.finalize()
    return nc


def _emit(nc, tc, xo_d, xf_d, wq_d, wk_d, wv_d, wo_d, bm_d, on_d, out_d):
    from contextlib import ExitStack

    exp_op = _get_exp_op()
    es = ExitStack()
    with es:
        sb = es.enter_context(tc.tile_pool(name="sb", bufs=2))
        sb3 = es.enter_context(tc.tile_pool(name="sb3", bufs=6))
        res = es.enter_context(tc.tile_pool(name="res", bufs=1))
        ps2 = es.enter_context(tc.tile_pool(name="ps2", bufs=2, space="PSUM"))

        # ---------------- resident tensors (wk/wv first: chunk 0 needs them)
        wk = res.tile([128, DT, HKV * HD], BF16, tag="wk")
        nc.sync.dma_start(wk[:], wk_d.rearrange("(dt p) h -> p dt h", p=128))
        wv = res.tile([128, DT, HKV * HD], BF16, tag="wv")
        nc.sync.dma_start(wv[:], wv_d.rearrange("(dt p) h -> p dt h", p=128))
        xo = res.tile([128, DT, LQ], BF16, tag="xo")          # x^T own cols
        bm = res.tile([128, 4, QB], BF16, tag="bm")          # band masks

        kt_sb = [res.tile([128, T], BF16, tag=f"kt{h2}", name=f"kt{h2}") for h2 in range(2)]
        v_sb = res.tile([128, NKT, HKV, HD + 1], BF16, tag="v")  # [V | 1]
        qg_sb = [
            res.tile([128, G, LQ], BF16, tag=f"qg{h2}", name=f"qg{h2}")
            for h2 in range(2)
        ]
        ctx_sb = res.tile([128, 8, LQ], BF16, tag="ctx")      # normalized ctx^T
        ones_sb = res.tile([1, HD], F32R, tag="ones")
        nc.sync.dma_start(ones_sb[:], on_d[:])
        c3_sb = res.tile([128, 1], F32, tag="c3")             # spilled C3
        nc.vector.memset(c3_sb[:], EXP_C3)
        bias_sb = res.tile([128, 1], F32, tag="abias")        # ACT exp bias
        nc.vector.memset(bias_sb[:], ACT_BIAS)
        nc.vector.memset(v_sb[:, :, :, HD : HD + 1], 1.0)

        # ---------------- P1: projections. KV chunk 0 first (its inputs are
        # small and DMA'd first), then Q (whose fat wqt DMA overlaps chunk 0),
        # then KV chunks 1-7.
        def kv_parts(c):
            """K/V projection for 512-token chunk c as 6 small emission parts
            (interleaved between attention groups to fill exp-wait stalls)."""
            xf = sb.tile([128, DT, 512], BF16, tag="xf", name=f"xf{c}")
            nc.sync.dma_start(
                xf[:],
                xf_d.rearrange("(dt p) t -> p dt t", p=128)[:, :, 512 * c : 512 * (c + 1)],
            )

            def k_part(h2):
                psk = ps2.tile([128, 512], F32, tag="scores", name="psk", bufs=2)
                for d in range(DT):
                    nc.tensor.matmul(
                        psk[:],
                        wk[:, d, 128 * h2 : 128 * (h2 + 1)],
                        xf[:, d, :],
                        start=(d == 0),
                        stop=(d == DT - 1),
                    )
                nc.scalar.copy(kt_sb[h2][:, 512 * c : 512 * (c + 1)], psk[:])

            def v_part(tq):
                kt = 4 * c + tq
                psv = ps2.tile([128, HKV * HD], F32, tag="scores", name="psv", bufs=2)
                for d in range(DT):
                    nc.tensor.matmul(
                        psv[:],
                        xf[:, d, 128 * tq : 128 * (tq + 1)],
                        wv[:, d, :],
                        start=(d == 0),
                        stop=(d == DT - 1),
                    )
                nc.vector.tensor_copy(
                    v_sb[:, kt, :, 0:HD],
                    psv.rearrange("p (h e) -> p h e", h=HKV),
                )

            return [lambda h2=h2: k_part(h2) for h2 in range(2)] + [
                lambda tq=tq: v_part(tq) for tq in range(4)
            ]

        def kv_chunk(c):
            for p in kv_parts(c):
                p()

        kv_chunk(0)
        kv_chunk(1)

        # Q^T projection, scaled by 128/(ln2*sqrt(HD)); its fat input DMAs
        # stream while chunk 0 computes
        nc.sync.dma_start(xo[:], xo_d.rearrange("(dt p) q -> p dt q", p=128))
        wqt = sb.tile([128, DT, HQ * HD], BF16, tag="wbig", name="wqt")
        nc.sync.dma_start(wqt[:], wq_d.rearrange("(dt p) h -> p dt h", p=128))
        nc.sync.dma_start(bm[:], bm_d.rearrange("k p f -> p k f"))
        for m in range(8):
            psq = ps2.tile([128, LQ], F32, tag="pacc", name=f"psq{m}", bufs=1)
            for d in range(DT):
                nc.tensor.matmul(
                    psq[:],
                    wqt[:, d, 128 * m : 128 * (m + 1)],
                    xo[:, d, :],
                    start=(d == 0),
                    stop=(d == DT - 1),
                )
            nc.vector.tensor_scalar_mul(
                qg_sb[m // 4][:, m % 4, :], psq[:], QSCALE
            )


        # ---------------- P2: attention over blocks.
        # Software-pipelined emission: group g+1's score matmuls are emitted
        # BEFORE group g's ctx matmuls, so the PE streams scores while the
        # exp engines (ACT/DVE) process the previous group — no per-group PE
        # stall, which keeps the HAM activity monitor at full clock. A
        # block's normalize is likewise emitted after the next block's first
        # score group.
        ctx_tiles = {}

        def emit_scores_exp(j, kp, h2, gidx):
            qsl = slice(QB * j, QB * (j + 1))
            s_ps = ps2.tile([128, 2, 2, G, QB], F32, tag="scores", bufs=2)
            for kt2 in range(2):
                kt = 2 * kp + kt2
                ksl = slice(128 * kt, 128 * (kt + 1))
                for hs in range(2):
                    # adjacent matmuls alternate row groups -> concurrent
                    nc.tensor.matmul(
                        s_ps[:, hs, kt2, :, :],
                        kt_sb[h2][64 * hs : 64 * hs + 64, ksl],
                        qg_sb[h2][64 * hs : 64 * hs + 64, :, qsl],
                        start=True, stop=True,
                        tile_position=(64 * hs, 0),
                    )
            pt = sb3.tile([128, 2, 2, G, QB], BF16, tag="pt")
            s_flat = s_ps.rearrange("p a b g q -> p (a b g q)")
            p_flat = pt.rearrange("p a b g q -> p (a b g q)")
            if (gidx % 8) < DVE_K:
                nc.vector._custom_dve(
                    exp_op, out=p_flat.bitcast(I16), in0=s_flat,
                    in1=c3_sb[:], s0=EXP_C0, s1=EXP_B128, imm2=EXP_S128,
                )
            else:
                nc.scalar.activation(
                    p_flat, s_flat, mybir.ActivationFunctionType.Exp,
                    bias=bias_sb[:], scale=ACT_SCALE,
                )
            if kp >= 2 * j:  # boundary quartet: causal mask
                par = kp - 2 * j
                msk = bm[:, 2 * par : 2 * par + 2, None, :].to_broadcast(
                    (128, 2, G, QB)
                )
                for hs in range(2):
                    nc.vector.tensor_mul(pt[:, hs], pt[:, hs], msk)
            return pt

        def emit_ctx(j, kp, h2, pt):
            nkp = 2 * (j + 1)
            for kt2 in range(2):
                kt = 2 * kp + kt2
                for hs in range(2):
                    kv = 2 * h2 + hs
                    # start=True only on the very first matmul into this psum
                    # tile (marks the whole bank pending-zero)
                    nc.tensor.matmul(
                        ctx_tiles[j][h2][:, 256 * hs : 256 * (hs + 1)],
                        v_sb[:, kt, kv, :],
                        pt[:, hs, kt2, :, :],
                        start=(kp == 0 and kt2 == 0 and hs == 0),
                        stop=(kp == nkp - 1 and kt2 == 1),
                        skip_group_check=True,
                    )

        def emit_normalize(j):
            ctx_ps = ctx_tiles[j]
            rs = sb.tile([1, 2, 8 * QB], F32R, tag="rs")
            for h2 in range(2):
                nc.vector.tensor_copy(rs[0:1, h2, :], ctx_ps[h2][HD : HD + 1, :])
            hi_st = sb.tile([64, 8, QB], BF16, tag="hist")
            for h2 in range(2):
                # broadcast rowsum over 64 partitions, then reciprocal there
                bc = ps2.tile([HD, 8 * QB], F32, tag="pacc", name="bc", bufs=1)
                nc.tensor.matmul(
                    bc[:], ones_sb[:], rs[0:1, h2, :], start=True, stop=True,
                )
                bcs = sb.tile([HD, 8 * QB], F32, tag="bcs")
                nc.vector.reciprocal_approx_fast(out=bcs[:], in_=bc[:])
                for hs in range(2):
                    for mq in range(4):
                        s = 4 * hs + mq
                        m = 4 * h2 + mq
                        ssl = slice(QB * s, QB * (s + 1))
                        if hs == 0:
                            nc.vector.tensor_mul(
                                ctx_sb[0:64, m, QB * j : QB * (j + 1)],
                                ctx_ps[h2][0:HD, ssl],
                                bcs[:, ssl],
                            )
                        else:
                            nc.vector.tensor_mul(
                                hi_st[:, m, :], ctx_ps[h2][0:HD, ssl], bcs[:, ssl]
                            )
            # partition-shift the odd-kv heads to partitions 64-127 (DMA)
            nc.sync.dma_start(
                ctx_sb[64:128, :, QB * j : QB * (j + 1)], hi_st[:]
            )

        # P3 prefetch: wot streams in during P2; out groups interleave below
        out_sb = res.tile([128, 4, D], F32, tag="osb")
        wot = sb.tile([128, 8, D], BF16, tag="wbig", name="wot")
        nc.sync.dma_start(wot[:], wo_d.rearrange("(m p) dcol -> p m dcol", p=128))

        def emit_p3(tt):
            # out rows [128*tt, 128*(tt+1)) depend only on blocks 2tt, 2tt+1
            for dc in range(2):
                pso = ps2.tile([128, 512], F32, tag="pacc",
                               name=f"pso{tt}_{dc}", bufs=1)
                for m in range(8):
                    nc.tensor.matmul(
                        pso[:],
                        ctx_sb[:, m, 128 * tt : 128 * (tt + 1)],
                        wot[:, m, 512 * dc : 512 * (dc + 1)],
                        start=(m == 0),
                        stop=(m == 7),
                    )
                nc.vector.tensor_copy(
                    out_sb[:, tt, 512 * dc : 512 * (dc + 1)], pso[:]
                )
            nc.sync.dma_start(
                out_d.rearrange("(tt p) dcol -> p tt dcol", p=128)[:, tt, :],
                out_sb[:, tt, :],
            )

        groups = [
            (j, kp, h2)
            for j in range(BPC) for kp in range(2 * (j + 1)) for h2 in range(2)
        ]
        pending = None
        part_q = []
        for gidx, (j, kp, h2) in enumerate(groups):
            if kp == 0 and h2 == 0:
                # force-drain parts of any chunk this block reads
                while part_q and part_q[0][0] <= j:
                    part_q.pop(0)[1]()
                # queue the next K/V projection chunk; its parts interleave
                # into the exp-bound attention stream at ~1 per 3 groups so
                # the projection work also pads the exp-bound tail blocks
                if j + 2 <= 7:
                    part_q.extend((j + 2, p) for p in kv_parts(j + 2))
                ctx_tiles[j] = [
                    ps2.tile([HD + 1, 8 * QB], F32, tag="ctx",
                             name=f"ctxps{h2}_{j}", bufs=3)
                    for h2 in range(2)
                ]
            pt = emit_scores_exp(j, kp, h2, gidx)
            if part_q:
                part_q.pop(0)[1]()
            if pending is not None:
                pj, pkp, ph2, ppt = pending
                emit_ctx(pj, pkp, ph2, ppt)
                if pkp == 2 * (pj + 1) - 1 and ph2 == 1:
                    emit_normalize(pj)
                    if pj % 2 == 1:
                        emit_p3(pj // 2)
            pending = (j, kp, h2, pt)
        pj, pkp, ph2, ppt = pending
        emit_ctx(pj, pkp, ph2, ppt)
        emit_normalize(pj)
        emit_p3(pj // 2)


def _install_ntff_hook():
    """Provide antenv.axon_hooks (absent from this image's antenv) so that
    run_bass_kernel_spmd(trace=True) can NTFF-profile via libaxon_pjrt."""
    import sys as _sys
    import types as _types

    if "antenv.axon_hooks" not in _sys.modules:
        import antenv as _antenv

        mod = _types.ModuleType("antenv.axon_hooks")
        mod._HOOK = None

        def _set(h, _m=mod):
            _m._HOOK = h

        def _get(_m=mod):
            return _m._HOOK

        mod.set_axon_ntff_profile_hook = _set
        mod.get_axon_ntff_profile_hook = _get
        _sys.modules["antenv.axon_hooks"] = mod
        _antenv.axon_hooks = mod
    mod = _sys.modules["antenv.axon_hooks"]
    if mod.get_axon_ntff_profile_hook() is None:
        import trn_agent_boot.trn_boot as _tb

        hook = _tb._ntff_profile_via_ctypes("/opt/axon/libaxon_pjrt.so")
        mod.set_axon_ntff_profile_hook(hook)
    from concourse import bass_utils as _bu

    _bu.upload_artifacts = lambda tmpdir: f"local://{tmpdir}"


# ---------------------------------------------------------------- host side
_NC_CACHE = None


def _get_nc():
    global _NC_CACHE
    if _NC_CACHE is None:
        _NC_CACHE = build_nc()
    return _NC_CACHE


def _prep_in_maps(x, Wq, Wk, Wv, Wo):
    xT = np.ascontiguousarray(x[0].T).astype(np.float32)          # [D, T]
    xT_bf = xT.astype(BF16NP)
    wq_perm = np.empty_like(Wq)
    wo_perm = np.empty_like(Wo)
    for m in range(8):
        wq_perm[:, 128 * m : 128 * m + 64] = Wq[:, 64 * LO[m] : 64 * LO[m] + 64]
        wq_perm[:, 128 * m + 64 : 128 * m + 128] = Wq[:, 64 * HI[m] : 64 * HI[m] + 64]
        wo_perm[128 * m : 128 * m + 64, :] = Wo[64 * LO[m] : 64 * LO[m] + 64, :]
        wo_perm[128 * m + 64 : 128 * m + 128, :] = Wo[64 * HI[m] : 64 * HI[m] + 64, :]
    maps = []
    for i in range(NC):
        cols = _local_cols(i)
        maps.append({
            "xT_own": np.ascontiguousarray(xT_bf[:, cols]),
            "xT_full": xT_bf,
            "Wq_perm": wq_perm.astype(BF16NP),
            "Wk_n": Wk.astype(BF16NP),
            "Wv_n": Wv.astype(BF16NP),
            "Wo_perm": wo_perm.astype(BF16NP),
            "bmask": _band_mask(i),
            "ones_c": np.ones((1, HD), np.float32),
        })
    return maps


def kernel(x, Wq, Wk, Wv, Wo):
    nc = _get_nc()
    maps = _prep_in_maps(
        np.asarray(x, np.float32),
        np.asarray(Wq, np.float32),
        np.asarray(Wk, np.float32),
        np.asarray(Wv, np.float32),
        np.asarray(Wo, np.float32),
    )
    trace = bool(int(os.environ.get("KERNEL_TRACE", "0")))
    if trace:
        try:
            _install_ntff_hook()
        except Exception as e:  # profiling is best-effort
            print(f"ntff hook install failed: {e}")
    r = run_bass_kernel_spmd(nc, maps, list(range(NC)), trace=trace)
    out = np.empty((B, T, D), np.float32)
    for i in range(NC):
        out[0, _local_cols(i), :] = r.results[i]["out_loc"]
    if trace:
        kernel.last_exec_time_ns = r.exec_time_ns
        kernel.last_results = r
    return out


# revision 23
# speedup vs baseline: 1.1741x; 1.1741x over previous
"""GQA causal attention (B=1, T=4096, D=1024, HQ=16, HKV=4, HD=64) on 8 trn2
NeuronCores via Bass/Tile.

Sharding: block-cyclic sequence-parallel. The 4096 query tokens are split into
64 blocks of 64; core i owns blocks {i, 8+i, ..., 56+i} (512 q tokens). Every
core runs the SAME program (SPMD): for its j-th block it processes k-tiles
[0, 4*(j+1)) — a core-independent conservative causal extent — and a
host-supplied per-core boundary mask zeroes the non-causal tail, so per-core
work is uniform AND balanced. K/V are projected redundantly per-core from a
bf16 copy of x^T (no collectives: an AllGather + its first-use barrier
measured ~85us of mostly-serial wall time, more than the ~50us of redundant
matmul work it saves).

Softmax exp is split across TWO engines so the ACT engine stops being the
pipeline bottleneck (it was ~160us of 363us):
  - ACT path: nc.scalar.activation(Exp) with the score scale folded into its
    free affine (scale=ln2/128, bias=ln K).
  - DVE path: a custom 8-stage vector op (EXP2B16_ANT) that computes the bf16
    BIT PATTERN of K*e^s directly: u = x + C1 (x = 128*s/ln2, folded into the
    Q cast scale); e = round-to-multiple-of-128(u) via the +/- 1.5*2^30 trick;
    f = u - e; bits = u + 0.316/128*f^2 + 53 (Schraudolph + quadratic mantissa
    correction), written as int16 and bitcast to bf16. Max rel err 0.59%, and
    the uniform factor K cancels in the softmax normalization ([V|1] rowsum
    uses the same pt values).
Boundary causal masks are applied by the otherwise-idle GPSIMD engine
(SBUF->SBUF bf16 multiplies by a 0/1 mask).

Layout strategy (avoids all on-device transposes):
  - scores are computed as S^T[k, q] with k on partitions, so the softmax
    denominator comes from a ones-column appended to the V stationary ([V|1]).
  - normalization is deferred: ctx^T/(rowsum) after the k-loop via a
    broadcast-matmul + fast reciprocal.
  - Wq/Wo columns/rows host-permuted so two heads stack into 128 partitions;
    score matmuls run pairwise-packed via tile_position row groups.
All weights/activations bf16 (FWL fast weight load + half the DMA/SBUF);
accumulation fp32 in PSUM.
"""

import math
import os
import sys

sys.path.insert(0, "/opt/trn_rl_repo")

import numpy as np
import ml_dtypes

import concourse.bass as bass
import concourse.bacc as bacc
import concourse.mybir as mybir
import concourse.tile as tile
from concourse.bass_utils import run_bass_kernel_spmd

# ---------------------------------------------------------------- constants
B, T, D = 1, 4096, 1024
HQ, HKV, HD = 16, 4, 64
G = HQ // HKV          # 4 q heads per kv head
NC = 8                 # cores
QB = 64                # q block size
NBLK = T // QB         # 64 blocks total
BPC = NBLK // NC       # 8 blocks per core
LQ = QB * BPC          # 512 local q tokens per core
DT = D // 128          # 8 contraction tiles over D
NKT = T // 128         # 32 k-tiles
F32 = mybir.dt.float32
F32R = mybir.dt.float32r
BF16 = mybir.dt.bfloat16
I16 = mybir.dt.int16
BF16NP = ml_dtypes.bfloat16

# exp split: scores arrive pre-scaled to x = 128*s/ln2 (folded into Q cast)
A128 = 128.0 / math.log(2.0)           # 184.665058...
QSCALE = A128 * (float(HD) ** -0.5)    # folded into the q bf16 cast
EXP_B128 = 16192.619648                # phase/level constant (calibrated)
EXP_C0 = -0.00247                      # -(mantissa curvature)/128
EXP_C3 = -22052.576                    # level decouple: C0*C3 = +54.47 bits
EXP_S128 = 1610612736.0                # 1.5*2^30: fp32 ulp=128 rounding trick
ACT_SCALE = math.log(2.0) / 128.0      # undoes the 128/ln2 prescale on ACT
ACT_BIAS = 0.009612216315190271        # ln K, K = DVE path's constant factor
# exp engine pattern: position g%8 -> 'D' (DVE) or 'A' (ACT)
EXP_PAT = os.environ.get("EXP_PAT", "DDDAAAAA")

# head pairing: pair tile m holds (LO[m] on partitions 0-63, HI[m] on 64-127)
LO = [0, 1, 2, 3, 8, 9, 10, 11]
HI = [4, 5, 6, 7, 12, 13, 14, 15]

# ------------------------------------------------- custom DVE exp operation
_EXP_OP = None


def _get_exp_op():
    """Register EXP2B16_ANT: out_bits16 = (u - ((f*f) - C3)*C0)*1 with
    u = in0 + C1, f = u - ((u + C2) - C2). Written as int16 (the bf16 bit
    pattern of K*e^s)."""
    global _EXP_OP
    if _EXP_OP is not None:
        return _EXP_OP
    from concourse.dve_spec import (
        C0, C1, C2, C3, Spec, Src0, lower, sq, _spill_c3_to_src1, _has_src1,
    )
    from concourse.dve_uop import DveOpSpec
    import concourse.dve_ops as dve_ops

    u = Src0 + C1
    w = u + C2
    e = w - C2
    f = u - e
    q = sq(f)
    r = q - C3
    p = r * C0
    body = u - p

    def _ref(in0, in1, s0, s1, imm2):
        # leaf binding: C0 <- s0 (curvature), C1 <- s1 (bias), C2 <- imm2,
        # C3 <- in1 (spilled, [P,1])
        x = in0.astype(np.float32)
        c3 = np.asarray(in1, np.float32).reshape(-1, *([1] * (x.ndim - 1)))
        uu = (x + np.float32(s1)).astype(np.float32)
        ee = ((uu + np.float32(imm2)) - np.float32(imm2)).astype(np.float32)
        ff = (uu - ee).astype(np.float32)
        pp = (((ff * ff) - c3) * np.float32(s0)).astype(np.float32)
        return (uu - pp).astype(np.float32)

    spec = Spec(body=_spill_c3_to_src1(body), reference=_ref)
    name = "EXP2B16_ANT"
    if name not in dve_ops._SUB_OPCODE_FOR_NAME:
        row = dve_ops._CUSTOM_DVE_ROW_BASE + len(dve_ops.OPS)
        dve_ops._SUB_OPCODE_FOR_NAME[name] = row
    shas = {}
    for ver in ("v3", "v4"):
        d = DveOpSpec(
            name=name,
            opcode=dve_ops._SUB_OPCODE_FOR_NAME[name],
            uops=lower(spec, ver=ver),
            rd1_en=_has_src1(spec),
        )
        shas[ver] = d.sha(ver)
    op = dve_ops.DveOp(name, spec, subdim=False, uops_sha=shas)
    if not any(o.name == name for o in dve_ops.OPS):
        dve_ops.OPS.append(op)
        dve_ops.CUSTOM_DVE_SPECS[name] = op.spec
    _EXP_OP = op
    return op


def _local_cols(i):
    """Global token indices owned by core i, in local order."""
    return np.concatenate(
        [QB * (NC * j + i) + np.arange(QB) for j in range(BPC)]
    )


def _band_mask(i):
    """[4, 128, 64] multiplicative causal mask for the last k-quartet of any
    block: valid iff 128*kt2 + p <= 64*i + f."""
    kt2 = np.arange(4)[:, None, None]
    p = np.arange(128)[None, :, None]
    f = np.arange(64)[None, None, :]
    return (128 * kt2 + p <= 64 * i + f).astype(BF16NP)


# ---------------------------------------------------------------- program
def build_nc():
    nc = bacc.Bacc(None)
    xo_d = nc.declare_dram_parameter("xT_own", [D, LQ], BF16, isOutput=False)
    xf_d = nc.declare_dram_parameter("xT_full", [D, T], BF16, isOutput=False)
    wq_d = nc.declare_dram_parameter("Wq_perm", [D, HQ * HD], BF16, isOutput=False)
    wk_d = nc.declare_dram_parameter("Wk_n", [D, HKV * HD], BF16, isOutput=False)
    wv_d = nc.declare_dram_parameter("Wv_n", [D, HKV * HD], BF16, isOutput=False)
    wo_d = nc.declare_dram_parameter("Wo_perm", [HQ * HD, D], BF16, isOutput=False)
    bm_d = nc.declare_dram_parameter("bmask", [4, 128, QB], BF16, isOutput=False)
    on_d = nc.declare_dram_parameter("ones_c", [1, HD], F32R, isOutput=False)
    out_d = nc.declare_dram_parameter("out_loc", [LQ, D], F32, isOutput=True)

    with tile.TileContext(nc) as tc:
        _emit(nc, tc, xo_d, xf_d, wq_d, wk_d, wv_d, wo_d, bm_d, on_d, out_d)
    nc.finalize()
    return nc


def _emit(nc, tc, xo_d, xf_d, wq_d, wk_d, wv_d, wo_d, bm_d, on_d, out_d):
    from contextlib import ExitStack

    exp_op = _get_exp_op()
    es = ExitStack()
    with es:
        sb = es.enter_context(tc.tile_pool(name="sb", bufs=2))
        sb3 = es.enter_context(tc.tile_pool(name="sb3", bufs=6))
        res = es.enter_context(tc.tile_pool(name="res", bufs=1))
        ps2 = es.enter_context(tc.tile_pool(name="ps2", bufs=2, space="PSUM"))

        # ---------------- resident tensors (wk/wv first: chunk 0 needs them)
        wk = res.tile([128, DT, HKV * HD], BF16, tag="wk")
        nc.sync.dma_start(wk[:], wk_d.rearrange("(dt p) h -> p dt h", p=128))
        wv = res.tile([128, DT, HKV * HD], BF16, tag="wv")
        nc.sync.dma_start(wv[:], wv_d.rearrange("(dt p) h -> p dt h", p=128))
        xo = res.tile([128, DT, LQ], BF16, tag="xo")          # x^T own cols
        bm = res.tile([128, 4, QB], BF16, tag="bm")          # band masks

        kt_sb = [res.tile([128, T], BF16, tag=f"kt{h2}", name=f"kt{h2}") for h2 in range(2)]
        v_sb = res.tile([128, NKT, HKV, HD + 1], BF16, tag="v")  # [V | 1]
        qg_sb = [
            res.tile([128, G, LQ], BF16, tag=f"qg{h2}", name=f"qg{h2}")
            for h2 in range(2)
        ]
        ctx_sb = res.tile([128, 8, LQ], BF16, tag="ctx")      # normalized ctx^T
        ones_sb = res.tile([1, HD], F32R, tag="ones")
        nc.sync.dma_start(ones_sb[:], on_d[:])
        c3_sb = res.tile([128, 1], F32, tag="c3")             # spilled C3
        nc.vector.memset(c3_sb[:], EXP_C3)
        bias_sb = res.tile([128, 1], F32, tag="abias")        # ACT exp bias
        nc.vector.memset(bias_sb[:], ACT_BIAS)
        nc.vector.memset(v_sb[:, :, :, HD : HD + 1], 1.0)

        # ---------------- P1: projections. KV chunk 0 first (its inputs are
        # small and DMA'd first), then Q (whose fat wqt DMA overlaps chunk 0),
        # then KV chunks 1-7.
        def kv_parts(c):
            """K/V projection for 512-token chunk c as 6 small emission parts
            (interleaved between attention groups to fill exp-wait stalls)."""
            xf = sb.tile([128, DT, 512], BF16, tag="xf", name=f"xf{c}")
            nc.sync.dma_start(
                xf[:],
                xf_d.rearrange("(dt p) t -> p dt t", p=128)[:, :, 512 * c : 512 * (c + 1)],
            )

            def k_part(h2):
                psk = ps2.tile([128, 512], F32, tag="scores", name="psk", bufs=2)
                for d in range(DT):
                    nc.tensor.matmul(
                        psk[:],
                        wk[:, d, 128 * h2 : 128 * (h2 + 1)],
                        xf[:, d, :],
                        start=(d == 0),
                        stop=(d == DT - 1),
                    )
                nc.scalar.copy(kt_sb[h2][:, 512 * c : 512 * (c + 1)], psk[:])

            def v_part(tq):
                kt = 4 * c + tq
                psv = ps2.tile([128, HKV * HD], F32, tag="scores", name="psv", bufs=2)
                for d in range(DT):
                    nc.tensor.matmul(
                        psv[:],
                        xf[:, d, 128 * tq : 128 * (tq + 1)],
                        wv[:, d, :],
                        start=(d == 0),
                        stop=(d == DT - 1),
                    )
                nc.vector.tensor_copy(
                    v_sb[:, kt, :, 0:HD],
                    psv.rearrange("p (h e) -> p h e", h=HKV),
                )

            return [lambda h2=h2: k_part(h2) for h2 in range(2)] + [
                lambda tq=tq: v_part(tq) for tq in range(4)
            ]

        def kv_chunk(c):
            for p in kv_parts(c):
                p()

        kv_chunk(0)
        kv_chunk(1)

        # Q^T projection, scaled by 128/(ln2*sqrt(HD)); its fat input DMAs
        # stream while chunk 0 computes
        nc.sync.dma_start(xo[:], xo_d.rearrange("(dt p) q -> p dt q", p=128))
        wqt = sb.tile([128, DT, HQ * HD], BF16, tag="wbig", name="wqt")
        nc.sync.dma_start(wqt[:], wq_d.rearrange("(dt p) h -> p dt h", p=128))
        nc.sync.dma_start(bm[:], bm_d.rearrange("k p f -> p k f"))
        for m in range(8):
            psq = ps2.tile([128, LQ], F32, tag="pacc", name=f"psq{m}", bufs=1)
            for d in range(DT):
                nc.tensor.matmul(
                    psq[:],
                    wqt[:, d, 128 * m : 128 * (m + 1)],
                    xo[:, d, :],
                    start=(d == 0),
                    stop=(d == DT - 1),
                )
            nc.vector.tensor_scalar_mul(
                qg_sb[m // 4][:, m % 4, :], psq[:], QSCALE
            )


        # ---------------- P2: attention over blocks.
        # Software-pipelined emission: group g+1's score matmuls are emitted
        # BEFORE group g's ctx matmuls, so the PE streams scores while the
        # exp engines (ACT/DVE) process the previous group — no per-group PE
        # stall, which keeps the HAM activity monitor at full clock. A
        # block's normalize is likewise emitted after the next block's first
        # score group.
        ctx_tiles = {}

        def emit_scores_exp(j, kp, h2, gidx):
            qsl = slice(QB * j, QB * (j + 1))
            s_ps = ps2.tile([128, 2, 2, G, QB], F32, tag="scores", bufs=2)
            for kt2 in range(2):
                kt = 2 * kp + kt2
                ksl = slice(128 * kt, 128 * (kt + 1))
                for hs in range(2):
                    # adjacent matmuls alternate row groups -> concurrent
                    nc.tensor.matmul(
                        s_ps[:, hs, kt2, :, :],
                        kt_sb[h2][64 * hs : 64 * hs + 64, ksl],
                        qg_sb[h2][64 * hs : 64 * hs + 64, :, qsl],
                        start=True, stop=True,
                        tile_position=(64 * hs, 0),
                    )
            pt = sb3.tile([128, 2, 2, G, QB], BF16, tag="pt")
            s_flat = s_ps.rearrange("p a b g q -> p (a b g q)")
            p_flat = pt.rearrange("p a b g q -> p (a b g q)")
            if EXP_PAT[gidx % len(EXP_PAT)] == "D":
                nc.vector._custom_dve(
                    exp_op, out=p_flat.bitcast(I16), in0=s_flat,
                    in1=c3_sb[:], s0=EXP_C0, s1=EXP_B128, imm2=EXP_S128,
                )
            else:
                nc.scalar.activation(
                    p_flat, s_flat, mybir.ActivationFunctionType.Exp,
                    bias=bias_sb[:], scale=ACT_SCALE,
                )
            if kp >= 2 * j:  # boundary quartet: causal mask
                par = kp - 2 * j
                msk = bm[:, 2 * par : 2 * par + 2, None, :].to_broadcast(
                    (128, 2, G, QB)
                )
                for hs in range(2):
                    nc.vector.tensor_mul(pt[:, hs], pt[:, hs], msk)
            return pt

        def emit_ctx(j, kp, h2, pt):
            nkp = 2 * (j + 1)
            for kt2 in range(2):
                kt = 2 * kp + kt2
                for hs in range(2):
                    kv = 2 * h2 + hs
                    # start=True only on the very first matmul into this psum
                    # tile (marks the whole bank pending-zero)
                    nc.tensor.matmul(
                        ctx_tiles[j][h2][:, 256 * hs : 256 * (hs + 1)],
                        v_sb[:, kt, kv, :],
                        pt[:, hs, kt2, :, :],
                        start=(kp == 0 and kt2 == 0 and hs == 0),
                        stop=(kp == nkp - 1 and kt2 == 1),
                        skip_group_check=True,
                    )

        def emit_normalize(j):
            ctx_ps = ctx_tiles[j]
            rs = sb.tile([1, 2, 8 * QB], F32R, tag="rs")
            for h2 in range(2):
                nc.vector.tensor_copy(rs[0:1, h2, :], ctx_ps[h2][HD : HD + 1, :])
            hi_st = sb.tile([64, 8, QB], BF16, tag="hist")
            for h2 in range(2):
                # broadcast rowsum over 64 partitions, then reciprocal there
                bc = ps2.tile([HD, 8 * QB], F32, tag="pacc", name="bc", bufs=1)
                nc.tensor.matmul(
                    bc[:], ones_sb[:], rs[0:1, h2, :], start=True, stop=True,
                )
                bcs = sb.tile([HD, 8 * QB], F32, tag="bcs")
                nc.vector.reciprocal_approx_fast(out=bcs[:], in_=bc[:])
                for hs in range(2):
                    for mq in range(4):
                        s = 4 * hs + mq
                        m = 4 * h2 + mq
                        ssl = slice(QB * s, QB * (s + 1))
                        if hs == 0:
                            nc.vector.tensor_mul(
                                ctx_sb[0:64, m, QB * j : QB * (j + 1)],
                                ctx_ps[h2][0:HD, ssl],
                                bcs[:, ssl],
                            )
                        else:
                            nc.vector.tensor_mul(
                                hi_st[:, m, :], ctx_ps[h2][0:HD, ssl], bcs[:, ssl]
                            )
            # partition-shift the odd-kv heads to partitions 64-127 (DMA)
            nc.sync.dma_start(
                ctx_sb[64:128, :, QB * j : QB * (j + 1)], hi_st[:]
            )

        # P3 prefetch: wot streams in during P2; out groups interleave below
        out_sb = res.tile([128, 4, D], F32, tag="osb")
        wot = sb.tile([128, 8, D], BF16, tag="wbig", name="wot")
        nc.sync.dma_start(wot[:], wo_d.rearrange("(m p) dcol -> p m dcol", p=128))

        def emit_p3(tt):
            # out rows [128*tt, 128*(tt+1)) depend only on blocks 2tt, 2tt+1
            for dc in range(2):
                pso = ps2.tile([128, 512], F32, tag="pacc",
                               name=f"pso{tt}_{dc}", bufs=1)
                for m in range(8):
                    nc.tensor.matmul(
                        pso[:],
                        ctx_sb[:, m, 128 * tt : 128 * (tt + 1)],
                        wot[:, m, 512 * dc : 512 * (dc + 1)],
                        start=(m == 0),
                        stop=(m == 7),
                    )
                nc.vector.tensor_copy(
                    out_sb[:, tt, 512 * dc : 512 * (dc + 1)], pso[:]
                )
            nc.sync.dma_start(
                out_d.rearrange("(tt p) dcol -> p tt dcol", p=128)[:, tt, :],
                out_sb[:, tt, :],
            )

        groups = [
            (j, kp, h2)
            for j in range(BPC) for kp in range(2 * (j + 1)) for h2 in range(2)
        ]
        pending = None
        part_q = []
        for gidx, (j, kp, h2) in enumerate(groups):
            if kp == 0 and h2 == 0:
                # force-drain parts of any chunk this block reads
                while part_q and part_q[0][0] <= j:
                    part_q.pop(0)[1]()
                # queue the next K/V projection chunk; its parts interleave
                # into the exp-bound attention stream at ~1 per 3 groups so
                # the projection work also pads the exp-bound tail blocks
                if j + 2 <= 7:
                    part_q.extend((j + 2, p) for p in kv_parts(j + 2))
                ctx_tiles[j] = [
                    ps2.tile([HD + 1, 8 * QB], F32, tag="ctx",
                             name=f"ctxps{h2}_{j}", bufs=3)
                    for h2 in range(2)
                ]
            pt = emit_scores_exp(j, kp, h2, gidx)
            if part_q:
                part_q.pop(0)[1]()
            if pending is not None:
                pj, pkp, ph2, ppt = pending
                emit_ctx(pj, pkp, ph2, ppt)
                if pkp == 2 * (pj + 1) - 1 and ph2 == 1:
                    emit_normalize(pj)
                    if pj % 2 == 1:
                        emit_p3(pj // 2)
            pending = (j, kp, h2, pt)
        pj, pkp, ph2, ppt = pending
        emit_ctx(pj, pkp, ph2, ppt)
        emit_normalize(pj)
        emit_p3(pj // 2)


def _install_ntff_hook():
    """Provide antenv.axon_hooks (absent from this image's antenv) so that
    run_bass_kernel_spmd(trace=True) can NTFF-profile via libaxon_pjrt."""
    import sys as _sys
    import types as _types

    if "antenv.axon_hooks" not in _sys.modules:
        import antenv as _antenv

        mod = _types.ModuleType("antenv.axon_hooks")
        mod._HOOK = None

        def _set(h, _m=mod):
            _m._HOOK = h

        def _get(_m=mod):
            return _m._HOOK

        mod.set_axon_ntff_profile_hook = _set
        mod.get_axon_ntff_profile_hook = _get
        _sys.modules["antenv.axon_hooks"] = mod
        _antenv.axon_hooks = mod
    mod = _sys.modules["antenv.axon_hooks"]
    if mod.get_axon_ntff_profile_hook() is None:
        import trn_agent_boot.trn_boot as _tb

        hook = _tb._ntff_profile_via_ctypes("/opt/axon/libaxon_pjrt.so")
        mod.set_axon_ntff_profile_hook(hook)
    from concourse import bass_utils as _bu

    _bu.upload_artifacts = lambda tmpdir: f"local://{tmpdir}"


# ---------------------------------------------------------------- host side
_NC_CACHE = None


def _get_nc():
    global _NC_CACHE
    if _NC_CACHE is None:
        _NC_CACHE = build_nc()
    return _NC_CACHE


def _prep_in_maps(x, Wq, Wk, Wv, Wo):
    xT = np.ascontiguousarray(x[0].T).astype(np.float32)          # [D, T]
    xT_bf = xT.astype(BF16NP)
    wq_perm = np.empty_like(Wq)
    wo_perm = np.empty_like(Wo)
    for m in range(8):
        wq_perm[:, 128 * m : 128 * m + 64] = Wq[:, 64 * LO[m] : 64 * LO[m] + 64]
        wq_perm[:, 128 * m + 64 : 128 * m + 128] = Wq[:, 64 * HI[m] : 64 * HI[m] + 64]
        wo_perm[128 * m : 128 * m + 64, :] = Wo[64 * LO[m] : 64 * LO[m] + 64, :]
        wo_perm[128 * m + 64 : 128 * m + 128, :] = Wo[64 * HI[m] : 64 * HI[m] + 64, :]
    maps = []
    for i in range(NC):
        cols = _local_cols(i)
        maps.append({
            "xT_own": np.ascontiguousarray(xT_bf[:, cols]),
            "xT_full": xT_bf,
            "Wq_perm": wq_perm.astype(BF16NP),
            "Wk_n": Wk.astype(BF16NP),
            "Wv_n": Wv.astype(BF16NP),
            "Wo_perm": wo_perm.astype(BF16NP),
            "bmask": _band_mask(i),
            "ones_c": np.ones((1, HD), np.float32),
        })
    return maps


def kernel(x, Wq, Wk, Wv, Wo):
    nc = _get_nc()
    maps = _prep_in_maps(
        np.asarray(x, np.float32),
        np.asarray(Wq, np.float32),
        np.asarray(Wk, np.float32),
        np.asarray(Wv, np.float32),
        np.asarray(Wo, np.float32),
    )
    trace = bool(int(os.environ.get("KERNEL_TRACE", "0")))
    if trace:
        try:
            _install_ntff_hook()
        except Exception as e:  # profiling is best-effort
            print(f"ntff hook install failed: {e}")
    r = run_bass_kernel_spmd(nc, maps, list(range(NC)), trace=trace)
    out = np.empty((B, T, D), np.float32)
    for i in range(NC):
        out[0, _local_cols(i), :] = r.results[i]["out_loc"]
    if trace:
        kernel.last_exec_time_ns = r.exec_time_ns
        kernel.last_results = r
    return out
